# revision 16
# baseline (speedup 1.0000x reference)
"""Adaptive-threshold spike encoding on 8 TRN2 NeuronCores.

Math: the reference scans t=0..31 with
    acc += x; spike = acc >= thr_t; acc = spike ? 0 : acc; thr' = 0.9*thr + 0.1*|x|
With thr_t = x + 0.9^t*(0.5-x) (closed form), spike_t <=> acc_pre >= 0.9^t*(0.5-x)
where acc_pre = k*x (k = steps since last reset).  Dividing by x and scaling by
g^t (g = 1/0.9):  spike_t <=> m >= r  with  r = (0.5-x)/x  and  m = k*g^t,
updated as m' = select(m < r, g*m + q_t, 0),  q_t = g^(t+1).

TWO timesteps fuse into ONE custom-DVE op (one uop, 1x rate):
    M1 = select(m < r, g*m + q_t, 0);  m2 = select(M1 < r, g*M1 + q_t1, 0)
The pair state m2 three-way classifies both spike bits (for r > 0):
    m2 == 0        -> odd-step spike   (even 0, odd 1)
    m2 == q_t1     -> even-step spike  (M1 was reset; g*0+q_t1 is exact)
    m2 >= 2*q_t1   -> no spikes
(r <= 0, i.e. x >= 0.5, always has m2 == 0 and both bits set; host knows x.)
ScalarE compresses m2 to a uint8 code via Exp(-m2/q_t1 + 2.2):
m2=0 -> e^2.2=9.0 {9};  m2=q_t1 -> e^1.2=3.3 {3};  m2>=2q_t1 -> <=e^0.2 {0,1}
(large m2 decays to +0 — no overflow path).  Host decodes: odd = code>=6;
even = (2<code<6) | (odd & (x>=0.5)).

Sharding: feature dim across the 8 cores, 8192 features each, no comms.
"""

import numpy as np
import concourse.bass as bass
import concourse.bacc as bacc
import concourse.mybir as mybir
from concourse import dve_ops as _dve_ops
from concourse.dve_spec import C0, C1, C2, Spec, Src0, Src1, Zero, select, lower
from concourse.dve_uop import DveOpSpec
from concourse.bass_utils import run_bass_kernel_spmd

B = 32
F = 65536
T = 32
NCORES = 8
FS = F // NCORES  # 8192 features per core
FH = 4
FL = FS // FH  # 2048
P = B * FH  # 128 partitions
NPAIR = T // 2  # 16 step-pairs

G = 1.0 / 0.9
RING = 4

_cache: dict = {}


def _register(name, body, reference):
    for op in _dve_ops.OPS:
        if op.name == name:
            return op
    spec = Spec(body=body, reference=reference)
    shas = {}
    for ver in ("v3", "v4"):
        uops = lower(spec, ver=ver)
        shas[ver] = DveOpSpec(name=name, opcode=0, uops=uops, rd1_en=True).sha(ver)
    op = _dve_ops.DveOp(name, spec, subdim=False, uops_sha=shas)
    _dve_ops.OPS.append(op)
    _dve_ops.CUSTOM_DVE_SPECS[name] = op.spec
    _dve_ops._SUB_OPCODE_FOR_NAME[name] = (
        _dve_ops._CUSTOM_DVE_ROW_BASE + len(_dve_ops.OPS) - 1
    )
    return op


def _nr_r_op():
    # r = (0.5 - x) * y1*(2 - x*y1)   — fused final Newton step + (0.5-x) mult
    # in0 = x, in1 = y1 (seed recip), s0 = 2.0, s1 = 0.5
    return _register(
        "RECIP_NR_R_ANT",
        (C1 - Src0) * ((C0 - Src0 * Src1) * Src1),
        lambda in0, in1, s0, s1, imm2: (
            (np.float32(s1) - in0.astype(np.float32))
            * ((np.float32(s0) - in0 * in1) * in1)
        ).astype(np.float32),
    )


def _state2_op():
    # in0 = m, in1 = r, s0 = g, s1 = q_t, imm2 = q_{t+1}
    S1 = select(Src0 < Src1, Src0 * C0 + C1, Zero)
    return _register(
        "SPIKE_STATE2_ANT",
        select(S1 < Src1, S1 * C0 + C2, Zero),
        lambda in0, in1, s0, s1, imm2: (
            lambda M1: np.where(M1 < in1, M1 * np.float32(s0) + np.float32(imm2), 0.0)
        )(
            np.where(
                in0 < in1, in0.astype(np.float32) * np.float32(s0) + np.float32(s1), 0.0
            ).astype(np.float32)
        ).astype(np.float32),
    )


def _build(repeat: int = 1) -> bass.Bass:
    f32 = mybir.dt.float32
    u8 = mybir.dt.uint8
    ALU = mybir.AluOpType
    op = _state2_op()
    nr_r = _nr_r_op()

    NP = NPAIR * repeat  # global step-pairs

    nc = bacc.Bacc(target_bir_lowering=False)
    x = nc.declare_dram_parameter("x", [B, FS], f32, isOutput=False)
    out = nc.declare_dram_parameter("out", [B, NPAIR, FS], u8, isOutput=True)

    with (
        nc.sbuf_tensor("x_sb", [P, FL], f32) as x_sb,
        nc.sbuf_tensor("d_sb", [P, FL], f32) as d_sb,
        nc.sbuf_tensor("inv_sb", [P, FL], f32) as inv_sb,
        nc.sbuf_tensor("r_sb", [P, FL], f32) as r_sb,
        nc.sbuf_tensor("mt0", [P, FL], f32) as mt0,
        nc.sbuf_tensor("mt1", [P, FL], f32) as mt1,
        nc.sbuf_tensor("mt2", [P, FL], f32) as mt2,
        nc.sbuf_tensor("mt3", [P, FL], f32) as mt3,
        nc.sbuf_tensor("cd0", [P, FL], u8) as cd0,
        nc.sbuf_tensor("cd1", [P, FL], u8) as cd1,
        nc.sbuf_tensor("cd2", [P, FL], u8) as cd2,
        nc.sbuf_tensor("cd3", [P, FL], u8) as cd3,
        nc.sbuf_tensor("bias_sb", [P, 1], f32) as bias_sb,
        nc.semaphore("sem_in") as sem_in,
        nc.semaphore("sem_m") as sem_m,
        nc.semaphore("sem_cd") as sem_cd,
        nc.semaphore("sem_out") as sem_out,
        nc.Block() as block,
    ):
        xv = x[:, :].rearrange("b (fh fl) -> (b fh) fl", fh=FH)
        ov = out[:, :, :].rearrange("b t (fh fl) -> t b fh fl", fh=FH)
        mts = [mt0, mt1, mt2, mt3]
        cds = [cd0, cd1, cd2, cd3]

        # pair p covers local steps 2p, 2p+1: q_even = g^(2p+1), q_odd = g^(2p+2)
        q_ev = [float(G ** ((2 * (p % NPAIR)) + 1)) for p in range(NP)]
        q_od = [float(G ** ((2 * (p % NPAIR)) + 2)) for p in range(NP)]
        last = NP - 1

        @block.sync
        def _(sync):
            sync.dma_start(out=x_sb[:, :], in_=xv).then_inc(sem_in, 16)
            for p in range(NP):
                sync.wait_ge(sem_cd, p + 1)
                sync.dma_start(
                    out=ov[p % NPAIR], in_=cds[p % RING][:, :]
                ).then_inc(sem_out, 16)

        @block.scalar
        def _(scalar):
            ACTF = mybir.ActivationFunctionType
            for p in range(NP):
                scalar.wait_ge(sem_m, p + 1)
                if p >= RING:
                    scalar.wait_ge(sem_out, 16 * (p - (RING - 1)))
                scalar.activation(
                    cds[p % RING][:, :],
                    mts[(p + 1) % RING][:, :],
                    ACTF.Exp,
                    bias=bias_sb[:, :],
                    scale=-1.0 / q_od[p],
                )
                scalar.drain().then_inc(sem_cd, 1)

        @block.vector
        def _(vector):
            # setup: r = (0.5 - x) / x, m = 0 (consumed only by this engine)
            vector.wait_ge(sem_in, 16)
            # clamp away exact zeros: reciprocal_approx seed is undefined at 0
            vector.tensor_scalar(x_sb[:, :], x_sb[:, :], 1e-35, None, ALU.max)
            vector.reciprocal_approx_fast(inv_sb[:, :], x_sb[:, :])
            vector._custom_dve(
                nr_r,
                out=r_sb[:, :],
                in0=x_sb[:, :],
                in1=inv_sb[:, :],
                s0=2.0,
                s1=0.5,
                imm2=0.0,
            )
            vector.tensor_scalar(mt0[:, :], x_sb[:, :], 0.0, None, ALU.mult)
            vector.memset(bias_sb[:, :], 2.2)
            vector.drain()

            for p in range(NP):
                if p >= RING:
                    # mt[(p+1)%RING] was read by gpsimd at pair p-RING
                    vector.wait_ge(sem_cd, p - (RING - 1))
                vector._custom_dve(
                    op,
                    out=mts[(p + 1) % RING][:, :],
                    in0=mts[p % RING][:, :],
                    in1=r_sb[:, :],
                    s0=G,
                    s1=q_ev[p],
                    imm2=q_od[p],
                )
                vector.drain().then_inc(sem_m, 1)
                if p % NPAIR == NPAIR - 1 and p != last:
                    # rep boundary (timing builds): restart the recurrence,
                    # but only after gpsimd consumed this pair's state.
                    vector.wait_ge(sem_cd, p + 1)
                    vector.tensor_scalar(
                        mts[(p + 1) % RING][:, :], x_sb[:, :], 0.0, None, ALU.mult
                    )
                    vector.drain()

    nc.finalize()
    return nc


def _get_nc(repeat: int = 1) -> bass.Bass:
    if repeat not in _cache:
        _cache[repeat] = _build(repeat)
    return _cache[repeat]


def _run(x: np.ndarray, repeat: int = 1):
    nc = _get_nc(repeat)
    shards = [
        np.ascontiguousarray(x[:, i * FS : (i + 1) * FS]) for i in range(NCORES)
    ]
    in_maps = [{"x": s} for s in shards]
    res = run_bass_kernel_spmd(nc, in_maps, core_ids=list(range(NCORES)))
    return [r["out"] for r in res.results]


def kernel(x: np.ndarray) -> np.ndarray:
    x = np.asarray(x, dtype=np.float32)
    outs = _run(x, repeat=1)
    code = np.concatenate(outs, axis=2)  # [B, NPAIR, F] uint8
    big = x >= 0.5  # r <= 0: spikes every step
    odd = code >= 6
    even = ((code > 2) & (code < 6)) | (odd & big[:, None, :])
    spikes = np.empty((B, T, F), dtype=np.float32)
    spikes[:, 0::2, :] = even
    spikes[:, 1::2, :] = odd
    return spikes


# revision 17
# speedup vs baseline: 1.9016x; 1.9016x over previous
"""Adaptive-threshold spike encoding on 8 TRN2 NeuronCores.

Math: the reference scans t=0..31 with
    acc += x; spike = acc >= thr_t; acc = spike ? 0 : acc; thr' = 0.9*thr + 0.1*|x|
With thr_t = x + 0.9^t*(0.5-x) (closed form), spike_t <=> acc_pre >= 0.9^t*(0.5-x)
where acc_pre = k*x (k = steps since last reset).  Dividing by x and scaling by
g^t (g = 1/0.9):  spike_t <=> m >= r  with  r = (0.5-x)/x  and  m = k*g^t,
updated as m' = select(m < r, g*m + q_t, 0),  q_t = g^(t+1).

TWO timesteps fuse into ONE custom-DVE op (one uop, 1x rate):
    M1 = select(m < r, g*m + q_t, 0);  m2 = select(M1 < r, g*M1 + q_t1, 0)
The pair state m2 three-way classifies both spike bits (for r > 0):
    m2 == 0        -> odd-step spike   (even 0, odd 1)
    m2 == q_t1     -> even-step spike  (M1 was reset; g*0+q_t1 is exact)
    m2 >= 2*q_t1   -> no spikes
(r <= 0, i.e. x >= 0.5, always has m2 == 0 and both bits set; host knows x.)
ScalarE compresses m2 to a uint8 code via Exp(-m2/q_t1 + 2.2):
m2=0 -> e^2.2=9.0 {9};  m2=q_t1 -> e^1.2=3.3 {3};  m2>=2q_t1 -> <=e^0.2 {0,1}
(large m2 decays to +0 — no overflow path).  Host decodes: odd = code>=6;
even = (2<code<6) | (odd & (x>=0.5)).

Sharding: feature dim across the 8 cores, 8192 features each, no comms.
"""

import numpy as np
import concourse.bass as bass
import concourse.bacc as bacc
import concourse.mybir as mybir
from concourse import dve_ops as _dve_ops
from concourse.dve_spec import C0, C1, C2, Spec, Src0, Src1, Zero, select, lower
from concourse.dve_uop import DveOpSpec
from concourse.bass_utils import run_bass_kernel_spmd

B = 32
F = 65536
T = 32
NCORES = 8
FS = F // NCORES  # 8192 features per core
FH = 4
FL = FS // FH  # 2048
P = B * FH  # 128 partitions
NPAIR = T // 2  # 16 step-pairs

G = 1.0 / 0.9
RING = 4

_cache: dict = {}


def _register(name, body, reference):
    for op in _dve_ops.OPS:
        if op.name == name:
            return op
    spec = Spec(body=body, reference=reference)
    shas = {}
    for ver in ("v3", "v4"):
        uops = lower(spec, ver=ver)
        shas[ver] = DveOpSpec(name=name, opcode=0, uops=uops, rd1_en=True).sha(ver)
    op = _dve_ops.DveOp(name, spec, subdim=False, uops_sha=shas)
    _dve_ops.OPS.append(op)
    _dve_ops.CUSTOM_DVE_SPECS[name] = op.spec
    _dve_ops._SUB_OPCODE_FOR_NAME[name] = (
        _dve_ops._CUSTOM_DVE_ROW_BASE + len(_dve_ops.OPS) - 1
    )
    return op


def _nr_r_op():
    # r = (0.5 - x) * y1*(2 - x*y1)   — fused final Newton step + (0.5-x) mult
    # in0 = x, in1 = y1 (seed recip), s0 = 2.0, s1 = 0.5
    return _register(
        "RECIP_NR_R_ANT",
        (C1 - Src0) * ((C0 - Src0 * Src1) * Src1),
        lambda in0, in1, s0, s1, imm2: (
            (np.float32(s1) - in0.astype(np.float32))
            * ((np.float32(s0) - in0 * in1) * in1)
        ).astype(np.float32),
    )


def _state2_op():
    # in0 = m, in1 = r, s0 = g, s1 = q_t, imm2 = q_{t+1}
    S1 = select(Src0 < Src1, Src0 * C0 + C1, Zero)
    return _register(
        "SPIKE_STATE2_ANT",
        select(S1 < Src1, S1 * C0 + C2, Zero),
        lambda in0, in1, s0, s1, imm2: (
            lambda M1: np.where(M1 < in1, M1 * np.float32(s0) + np.float32(imm2), 0.0)
        )(
            np.where(
                in0 < in1, in0.astype(np.float32) * np.float32(s0) + np.float32(s1), 0.0
            ).astype(np.float32)
        ).astype(np.float32),
    )


def _build(repeat: int = 1) -> bass.Bass:
    f32 = mybir.dt.float32
    u8 = mybir.dt.uint8
    ALU = mybir.AluOpType
    op = _state2_op()
    nr_r = _nr_r_op()

    NP = NPAIR * repeat  # global step-pairs

    nc = bacc.Bacc(target_bir_lowering=False)
    x = nc.declare_dram_parameter("x", [B, FS], f32, isOutput=False)
    out = nc.declare_dram_parameter("out", [B, NPAIR, FS], u8, isOutput=True)

    with (
        nc.sbuf_tensor("x_sb", [P, FL], f32) as x_sb,
        nc.sbuf_tensor("d_sb", [P, FL], f32) as d_sb,
        nc.sbuf_tensor("inv_sb", [P, FL], f32) as inv_sb,
        nc.sbuf_tensor("r_sb", [P, FL], f32) as r_sb,
        nc.sbuf_tensor("mt0", [P, FL], f32) as mt0,
        nc.sbuf_tensor("mt1", [P, FL], f32) as mt1,
        nc.sbuf_tensor("mt2", [P, FL], f32) as mt2,
        nc.sbuf_tensor("mt3", [P, FL], f32) as mt3,
        nc.sbuf_tensor("cd0", [P, FL], u8) as cd0,
        nc.sbuf_tensor("cd1", [P, FL], u8) as cd1,
        nc.sbuf_tensor("cd2", [P, FL], u8) as cd2,
        nc.sbuf_tensor("cd3", [P, FL], u8) as cd3,
        nc.sbuf_tensor("bias_sb", [P, 1], f32) as bias_sb,
        nc.semaphore("sem_in") as sem_in,
        nc.semaphore("sem_m") as sem_m,
        nc.semaphore("sem_cd") as sem_cd,
        nc.semaphore("sem_out") as sem_out,
        nc.Block() as block,
    ):
        xv = x[:, :].rearrange("b (fh fl) -> (b fh) fl", fh=FH)
        ov = out[:, :, :].rearrange("b t (fh fl) -> t b fh fl", fh=FH)
        mts = [mt0, mt1, mt2, mt3]
        cds = [cd0, cd1, cd2, cd3]

        # pair p covers local steps 2p, 2p+1: q_even = g^(2p+1), q_odd = g^(2p+2)
        q_ev = [float(G ** ((2 * (p % NPAIR)) + 1)) for p in range(NP)]
        q_od = [float(G ** ((2 * (p % NPAIR)) + 2)) for p in range(NP)]
        last = NP - 1

        @block.sync
        def _(sync):
            sync.dma_start(out=x_sb[:, :], in_=xv).then_inc(sem_in, 16)
            for p in range(NP):
                sync.wait_ge(sem_cd, p + 1)
                sync.dma_start(
                    out=ov[p % NPAIR], in_=cds[p % RING][:, :]
                ).then_inc(sem_out, 16)

        @block.scalar
        def _(scalar):
            ACTF = mybir.ActivationFunctionType
            # dummy activation: pulls the exp table load off the critical
            # path (overlaps the input DMA / DVE setup)
            scalar.activation(
                cds[RING - 1][:, :1],
                mts[RING - 1][:, :1],
                ACTF.Exp,
                bias=bias_sb[:, :],
                scale=0.0,
            )
            for p in range(NP):
                scalar.wait_ge(sem_m, p + 1)
                if p >= RING:
                    scalar.wait_ge(sem_out, 16 * (p - (RING - 1)))
                scalar.activation(
                    cds[p % RING][:, :],
                    mts[(p + 1) % RING][:, :],
                    ACTF.Exp,
                    bias=bias_sb[:, :],
                    scale=-1.0 / q_od[p],
                )
                scalar.drain().then_inc(sem_cd, 1)

        @block.vector
        def _(vector):
            # setup: r = (0.5 - x) / x, m = 0 (consumed only by this engine)
            vector.wait_ge(sem_in, 16)
            # clamp away exact zeros: reciprocal_approx seed is undefined at 0
            vector.tensor_scalar(x_sb[:, :], x_sb[:, :], 1e-35, None, ALU.max)
            vector.reciprocal_approx_fast(inv_sb[:, :], x_sb[:, :])
            vector._custom_dve(
                nr_r,
                out=r_sb[:, :],
                in0=x_sb[:, :],
                in1=inv_sb[:, :],
                s0=2.0,
                s1=0.5,
                imm2=0.0,
            )
            vector.tensor_scalar(mt0[:, :], x_sb[:, :], 0.0, None, ALU.mult)
            vector.memset(bias_sb[:, :], 2.2)
            vector.drain()

            for p in range(NP):
                if p >= RING:
                    # mt[(p+1)%RING] was read by gpsimd at pair p-RING
                    vector.wait_ge(sem_cd, p - (RING - 1))
                vector._custom_dve(
                    op,
                    out=mts[(p + 1) % RING][:, :],
                    in0=mts[p % RING][:, :],
                    in1=r_sb[:, :],
                    s0=G,
                    s1=q_ev[p],
                    imm2=q_od[p],
                )
                vector.drain().then_inc(sem_m, 1)
                if p % NPAIR == NPAIR - 1 and p != last:
                    # rep boundary (timing builds): restart the recurrence,
                    # but only after gpsimd consumed this pair's state.
                    vector.wait_ge(sem_cd, p + 1)
                    vector.tensor_scalar(
                        mts[(p + 1) % RING][:, :], x_sb[:, :], 0.0, None, ALU.mult
                    )
                    vector.drain()

    nc.finalize()
    return nc


def _get_nc(repeat: int = 1) -> bass.Bass:
    if repeat not in _cache:
        _cache[repeat] = _build(repeat)
    return _cache[repeat]


def _run(x: np.ndarray, repeat: int = 1):
    nc = _get_nc(repeat)
    shards = [
        np.ascontiguousarray(x[:, i * FS : (i + 1) * FS]) for i in range(NCORES)
    ]
    in_maps = [{"x": s} for s in shards]
    res = run_bass_kernel_spmd(nc, in_maps, core_ids=list(range(NCORES)))
    return [r["out"] for r in res.results]


def kernel(x: np.ndarray) -> np.ndarray:
    x = np.asarray(x, dtype=np.float32)
    outs = _run(x, repeat=1)
    code = np.concatenate(outs, axis=2)  # [B, NPAIR, F] uint8
    big = x >= 0.5  # r <= 0: spikes every step
    odd = code >= 6
    even = ((code > 2) & (code < 6)) | (odd & big[:, None, :])
    spikes = np.empty((B, T, F), dtype=np.float32)
    spikes[:, 0::2, :] = even
    spikes[:, 1::2, :] = odd
    return spikes


# revision 19
# speedup vs baseline: 2.4241x; 1.2747x over previous
"""Adaptive-threshold spike encoding on 8 TRN2 NeuronCores.

Math: the reference scans t=0..31 with
    acc += x; spike = acc >= thr_t; acc = spike ? 0 : acc; thr' = 0.9*thr + 0.1*|x|
With thr_t = x + 0.9^t*(0.5-x) (closed form), spike_t <=> acc_pre >= 0.9^t*(0.5-x)
where acc_pre = k*x (k = steps since last reset).  Dividing by x and scaling by
g^t (g = 1/0.9):  spike_t <=> m >= r  with  r = (0.5-x)/x  and  m = k*g^t,
updated as m' = select(m < r, g*m + q_t, 0),  q_t = g^(t+1).

TWO timesteps fuse into ONE custom-DVE op (one uop, 1x rate):
    M1 = select(m < r, g*m + q_t, 0);  m2 = select(M1 < r, g*M1 + q_t1, 0)
The pair state m2 three-way classifies both spike bits (for r > 0):
    m2 == 0        -> odd-step spike   (even 0, odd 1)
    m2 == q_t1     -> even-step spike  (M1 was reset; g*0+q_t1 is exact)
    m2 >= 2*q_t1   -> no spikes
(r <= 0, i.e. x >= 0.5, always has m2 == 0 and both bits set; host knows x.)
ScalarE compresses m2 to a uint8 code via Exp(-m2/q_t1 + 2.2):
m2=0 -> e^2.2=9.0 {9};  m2=q_t1 -> e^1.2=3.3 {3};  m2>=2q_t1 -> <=e^0.2 {0,1}
(large m2 decays to +0 — no overflow path).  Host decodes: odd = code>=6;
even = (2<code<6) | (odd & (x>=0.5)).

Sharding: feature dim across the 8 cores, 8192 features each, no comms.
"""

import numpy as np
import concourse.bass as bass
import concourse.bacc as bacc
import concourse.mybir as mybir
from concourse import dve_ops as _dve_ops
from concourse.dve_spec import (
    C0, C1, C2, Spec, Src0, Src1, Zero, select, lower, minn, _has_src1,
)
from concourse.dve_uop import DveOpSpec
from concourse.bass_utils import run_bass_kernel_spmd

B = 32
F = 65536
T = 32
NCORES = 8
FS = F // NCORES  # 8192 features per core
FH = 4
FL = FS // FH  # 2048
P = B * FH  # 128 partitions
NPAIR = T // 2  # 16 step-pairs

G = 1.0 / 0.9
RING = 4

_cache: dict = {}


def _register(name, body, reference):
    for op in _dve_ops.OPS:
        if op.name == name:
            return op
    spec = Spec(body=body, reference=reference)
    shas = {}
    for ver in ("v3", "v4"):
        uops = lower(spec, ver=ver)
        shas[ver] = DveOpSpec(
            name=name, opcode=0, uops=uops, rd1_en=_has_src1(spec)
        ).sha(ver)
    op = _dve_ops.DveOp(name, spec, subdim=False, uops_sha=shas)
    _dve_ops.OPS.append(op)
    _dve_ops.CUSTOM_DVE_SPECS[name] = op.spec
    _dve_ops._SUB_OPCODE_FOR_NAME[name] = (
        _dve_ops._CUSTOM_DVE_ROW_BASE + len(_dve_ops.OPS) - 1
    )
    return op


def _nr_r_op():
    # r = min((0.5 - x) * y1*(2 - x*y1), 3e38) — fused Newton step + (0.5-x)
    # mult; the min maps a NaN from an x==0 seed to "never spikes" (DVE
    # min/max pick the non-NaN operand).
    # in0 = x, in1 = y1 (seed recip), s0 = 2.0, s1 = 0.5, imm2 = 3e38
    return _register(
        "RECIP_NR_R2_ANT",
        minn((C1 - Src0) * ((C0 - Src0 * Src1) * Src1), C2),
        lambda in0, in1, s0, s1, imm2: np.minimum(
            np.nan_to_num(
                (np.float32(s1) - in0.astype(np.float32))
                * ((np.float32(s0) - in0 * in1) * in1),
                nan=np.float32(imm2),
            ),
            np.float32(imm2),
        ).astype(np.float32),
    )


def _first_pair_op():
    # pair from a zero state, reading only r: M1 = select(0 < r, q_t, 0);
    # out = select(M1 < r, g*M1 + q_t1, 0).  in0 = r, s0 = g, s1 = q_t,
    # imm2 = q_t1.
    M1 = select(Zero < Src0, C1, Zero)
    return _register(
        "SPIKE_FIRST_PAIR_ANT",
        select(M1 < Src0, M1 * C0 + C2, Zero),
        lambda in0, in1, s0, s1, imm2: (
            lambda M1: np.where(
                M1 < in0, M1 * np.float32(s0) + np.float32(imm2), 0.0
            )
        )(np.where(0.0 < in0, np.float32(s1), 0.0).astype(np.float32)).astype(
            np.float32
        ),
    )


def _state2_op():
    # in0 = m, in1 = r, s0 = g, s1 = q_t, imm2 = q_{t+1}
    S1 = select(Src0 < Src1, Src0 * C0 + C1, Zero)
    return _register(
        "SPIKE_STATE2_ANT",
        select(S1 < Src1, S1 * C0 + C2, Zero),
        lambda in0, in1, s0, s1, imm2: (
            lambda M1: np.where(M1 < in1, M1 * np.float32(s0) + np.float32(imm2), 0.0)
        )(
            np.where(
                in0 < in1, in0.astype(np.float32) * np.float32(s0) + np.float32(s1), 0.0
            ).astype(np.float32)
        ).astype(np.float32),
    )


def _build(repeat: int = 1) -> bass.Bass:
    f32 = mybir.dt.float32
    u8 = mybir.dt.uint8
    ALU = mybir.AluOpType
    op = _state2_op()
    nr_r = _nr_r_op()
    op0 = _first_pair_op()

    NP = NPAIR * repeat  # global step-pairs

    nc = bacc.Bacc(target_bir_lowering=False)
    x = nc.declare_dram_parameter("x", [B, FS], f32, isOutput=False)
    out = nc.declare_dram_parameter("out", [B, NPAIR, FS], u8, isOutput=True)

    with (
        nc.sbuf_tensor("x_sb", [P, FL], f32) as x_sb,
        nc.sbuf_tensor("d_sb", [P, FL], f32) as d_sb,
        nc.sbuf_tensor("inv_sb", [P, FL], f32) as inv_sb,
        nc.sbuf_tensor("r_sb", [P, FL], f32) as r_sb,
        nc.sbuf_tensor("mt0", [P, FL], f32) as mt0,
        nc.sbuf_tensor("mt1", [P, FL], f32) as mt1,
        nc.sbuf_tensor("mt2", [P, FL], f32) as mt2,
        nc.sbuf_tensor("mt3", [P, FL], f32) as mt3,
        nc.sbuf_tensor("cd0", [P, FL], u8) as cd0,
        nc.sbuf_tensor("cd1", [P, FL], u8) as cd1,
        nc.sbuf_tensor("cd2", [P, FL], u8) as cd2,
        nc.sbuf_tensor("cd3", [P, FL], u8) as cd3,
        nc.sbuf_tensor("bias_sb", [P, 1], f32) as bias_sb,
        nc.semaphore("sem_in") as sem_in,
        nc.semaphore("sem_m") as sem_m,
        nc.semaphore("sem_cd") as sem_cd,
        nc.semaphore("sem_out") as sem_out,
        nc.Block() as block,
    ):
        xv = x[:, :].rearrange("b (fh fl) -> (b fh) fl", fh=FH)
        ov = out[:, :, :].rearrange("b t (fh fl) -> t b fh fl", fh=FH)
        mts = [mt0, mt1, mt2, mt3]
        cds = [cd0, cd1, cd2, cd3]

        # pair p covers local steps 2p, 2p+1: q_even = g^(2p+1), q_odd = g^(2p+2)
        q_ev = [float(G ** ((2 * (p % NPAIR)) + 1)) for p in range(NP)]
        q_od = [float(G ** ((2 * (p % NPAIR)) + 2)) for p in range(NP)]
        last = NP - 1

        @block.sync
        def _(sync):
            sync.dma_start(out=x_sb[:, :], in_=xv).then_inc(sem_in, 16)
            for p in range(NP):
                sync.wait_ge(sem_cd, p + 1)
                sync.dma_start(
                    out=ov[p % NPAIR], in_=cds[p % RING][:, :]
                ).then_inc(sem_out, 16)

        @block.scalar
        def _(scalar):
            ACTF = mybir.ActivationFunctionType
            # dummy activation: pulls the exp table load off the critical
            # path (overlaps the input DMA / DVE setup)
            scalar.activation(
                cds[RING - 1][:, :1],
                mts[RING - 1][:, :1],
                ACTF.Exp,
                bias=bias_sb[:, :],
                scale=0.0,
            )
            for p in range(NP):
                scalar.wait_ge(sem_m, p + 2)
                if p >= RING:
                    scalar.wait_ge(sem_out, 16 * (p - (RING - 1)))
                scalar.activation(
                    cds[p % RING][:, :],
                    mts[(p + 1) % RING][:, :],
                    ACTF.Exp,
                    bias=bias_sb[:, :],
                    scale=-1.0 / q_od[p],
                )
                scalar.drain().then_inc(sem_cd, 1)

        @block.vector
        def _(vector):
            # setup: r = (0.5 - x) / x, m = 0 (consumed only by this engine)
            vector.memset(bias_sb[:, :], 2.2)
            vector.wait_ge(sem_in, 16)
            vector.reciprocal_approx_fast(inv_sb[:, :], x_sb[:, :])
            vector._custom_dve(
                nr_r,
                out=r_sb[:, :],
                in0=x_sb[:, :],
                in1=inv_sb[:, :],
                s0=2.0,
                s1=0.5,
                imm2=3e38,
            )
            vector.drain()

            for p in range(NP):
                if p >= RING:
                    # mt[(p+1)%RING] was read by ACT at pair p-RING
                    vector.wait_ge(sem_cd, p - (RING - 1))
                if p % NPAIR == 0:
                    # zero-state pair: reads only r, no state tile needed
                    vector._custom_dve(
                        op0,
                        out=mts[(p + 1) % RING][:, :],
                        in0=r_sb[:, :],
                        s0=G,
                        s1=q_ev[p],
                        imm2=q_od[p],
                    ).then_inc(sem_m, 1)
                else:
                    vector._custom_dve(
                        op,
                        out=mts[(p + 1) % RING][:, :],
                        in0=mts[p % RING][:, :],
                        in1=r_sb[:, :],
                        s0=G,
                        s1=q_ev[p],
                        imm2=q_od[p],
                    ).then_inc(sem_m, 1)
            # sem_m fires at op completion (pre-drain); ACT therefore waits
            # one op deeper, and this trailing drain covers the last pair.
            vector.drain().then_inc(sem_m, 1)

    nc.finalize()
    return nc


def _get_nc(repeat: int = 1) -> bass.Bass:
    if repeat not in _cache:
        _cache[repeat] = _build(repeat)
    return _cache[repeat]


def _run(x: np.ndarray, repeat: int = 1):
    nc = _get_nc(repeat)
    shards = [
        np.ascontiguousarray(x[:, i * FS : (i + 1) * FS]) for i in range(NCORES)
    ]
    in_maps = [{"x": s} for s in shards]
    res = run_bass_kernel_spmd(nc, in_maps, core_ids=list(range(NCORES)))
    return [r["out"] for r in res.results]


def kernel(x: np.ndarray) -> np.ndarray:
    x = np.asarray(x, dtype=np.float32)
    outs = _run(x, repeat=1)
    code = np.concatenate(outs, axis=2)  # [B, NPAIR, F] uint8
    big = x >= 0.5  # r <= 0: spikes every step
    odd = code >= 6
    even = ((code > 2) & (code < 6)) | (odd & big[:, None, :])
    spikes = np.empty((B, T, F), dtype=np.float32)
    spikes[:, 0::2, :] = even
    spikes[:, 1::2, :] = odd
    return spikes


# revision 21
# speedup vs baseline: 2.8673x; 1.1829x over previous
"""Adaptive-threshold spike encoding on 8 TRN2 NeuronCores.

Math: the reference scans t=0..31 with
    acc += x; spike = acc >= thr_t; acc = spike ? 0 : acc; thr' = 0.9*thr + 0.1*|x|
With thr_t = x + 0.9^t*(0.5-x) (closed form), spike_t <=> acc_pre >= 0.9^t*(0.5-x)
where acc_pre = k*x (k = steps since last reset).  Dividing by x and scaling by
g^t (g = 1/0.9):  spike_t <=> m >= r  with  r = (0.5-x)/x  and  m = k*g^t,
updated as m' = select(m < r, g*m + q_t, 0),  q_t = g^(t+1).

TWO timesteps fuse into ONE custom-DVE op (one uop, 1x rate):
    M1 = select(m < r, g*m + q_t, 0);  m2 = select(M1 < r, g*M1 + q_t1, 0)
The pair state m2 three-way classifies both spike bits (for r > 0):
    m2 == 0        -> odd-step spike   (even 0, odd 1)
    m2 == q_t1     -> even-step spike  (M1 was reset; g*0+q_t1 is exact)
    m2 >= 2*q_t1   -> no spikes
(r <= 0, i.e. x >= 0.5, always has m2 == 0 and both bits set; host knows x.)
ScalarE compresses m2 to a uint8 code via Exp(-m2/q_t1 + 2.2):
m2=0 -> e^2.2=9.0 {9};  m2=q_t1 -> e^1.2=3.3 {3};  m2>=2q_t1 -> <=e^0.2 {0,1}
(large m2 decays to +0 — no overflow path).  Host decodes: odd = code>=6;
even = (2<code<6) | (odd & (x>=0.5)).

Sharding: feature dim across the 8 cores, 8192 features each, no comms.
"""

import numpy as np
import concourse.bass as bass
import concourse.bacc as bacc
import concourse.mybir as mybir
from concourse import dve_ops as _dve_ops
from concourse.dve_spec import (
    C0, C1, C2, Spec, Src0, Src1, Zero, select, lower, minn, _has_src1,
)
from concourse.dve_uop import DveOpSpec
from concourse.bass_utils import run_bass_kernel_spmd

B = 32
F = 65536
T = 32
NCORES = 8
FS = F // NCORES  # 8192 features per core
FH = 4
FL = FS // FH  # 2048
P = B * FH  # 128 partitions
NPAIR = T // 2  # 16 step-pairs

G = 1.0 / 0.9
RING = 4

_cache: dict = {}


def _register(name, body, reference):
    for op in _dve_ops.OPS:
        if op.name == name:
            return op
    spec = Spec(body=body, reference=reference)
    shas = {}
    for ver in ("v3", "v4"):
        uops = lower(spec, ver=ver)
        shas[ver] = DveOpSpec(
            name=name, opcode=0, uops=uops, rd1_en=_has_src1(spec)
        ).sha(ver)
    op = _dve_ops.DveOp(name, spec, subdim=False, uops_sha=shas)
    _dve_ops.OPS.append(op)
    _dve_ops.CUSTOM_DVE_SPECS[name] = op.spec
    _dve_ops._SUB_OPCODE_FOR_NAME[name] = (
        _dve_ops._CUSTOM_DVE_ROW_BASE + len(_dve_ops.OPS) - 1
    )
    return op


def _nr_r_op():
    # r = min((0.5 - x) * y1*(2 - x*y1), 3e38) — fused Newton step + (0.5-x)
    # mult; the min maps a NaN from an x==0 seed to "never spikes" (DVE
    # min/max pick the non-NaN operand).
    # in0 = x, in1 = y1 (seed recip), s0 = 2.0, s1 = 0.5, imm2 = 3e38
    return _register(
        "RECIP_NR_R2_ANT",
        minn((C1 - Src0) * ((C0 - Src0 * Src1) * Src1), C2),
        lambda in0, in1, s0, s1, imm2: np.minimum(
            np.nan_to_num(
                (np.float32(s1) - in0.astype(np.float32))
                * ((np.float32(s0) - in0 * in1) * in1),
                nan=np.float32(imm2),
            ),
            np.float32(imm2),
        ).astype(np.float32),
    )


def _first_pair_op():
    # pair from a zero state, reading only r: M1 = select(0 < r, q_t, 0);
    # out = select(M1 < r, g*M1 + q_t1, 0).  in0 = r, s0 = g, s1 = q_t,
    # imm2 = q_t1.
    M1 = select(Zero < Src0, C1, Zero)
    return _register(
        "SPIKE_FIRST_PAIR_ANT",
        select(M1 < Src0, M1 * C0 + C2, Zero),
        lambda in0, in1, s0, s1, imm2: (
            lambda M1: np.where(
                M1 < in0, M1 * np.float32(s0) + np.float32(imm2), 0.0
            )
        )(np.where(0.0 < in0, np.float32(s1), 0.0).astype(np.float32)).astype(
            np.float32
        ),
    )


def _state2_op():
    # in0 = m, in1 = r, s0 = g, s1 = q_t, imm2 = q_{t+1}
    S1 = select(Src0 < Src1, Src0 * C0 + C1, Zero)
    return _register(
        "SPIKE_STATE2_ANT",
        select(S1 < Src1, S1 * C0 + C2, Zero),
        lambda in0, in1, s0, s1, imm2: (
            lambda M1: np.where(M1 < in1, M1 * np.float32(s0) + np.float32(imm2), 0.0)
        )(
            np.where(
                in0 < in1, in0.astype(np.float32) * np.float32(s0) + np.float32(s1), 0.0
            ).astype(np.float32)
        ).astype(np.float32),
    )


def _build(repeat: int = 1) -> bass.Bass:
    f32 = mybir.dt.float32
    u8 = mybir.dt.uint8
    ALU = mybir.AluOpType
    op = _state2_op()
    nr_r = _nr_r_op()
    op0 = _first_pair_op()

    NP = NPAIR * repeat  # global step-pairs

    nc = bacc.Bacc(target_bir_lowering=False)
    x = nc.declare_dram_parameter("x", [B, FS], f32, isOutput=False)
    out = nc.declare_dram_parameter("out", [B, NPAIR, FS], u8, isOutput=True)

    with (
        nc.sbuf_tensor("x_sb", [P, FL], f32) as x_sb,
        nc.sbuf_tensor("d_sb", [P, FL], f32) as d_sb,
        nc.sbuf_tensor("inv_sb", [P, FL], f32) as inv_sb,
        nc.sbuf_tensor("r_sb", [P, FL], f32) as r_sb,
        nc.sbuf_tensor("mt0", [P, FL], f32) as mt0,
        nc.sbuf_tensor("mt1", [P, FL], f32) as mt1,
        nc.sbuf_tensor("mt2", [P, FL], f32) as mt2,
        nc.sbuf_tensor("mt3", [P, FL], f32) as mt3,
        nc.sbuf_tensor("cd0", [P, FL], u8) as cd0,
        nc.sbuf_tensor("cd1", [P, FL], u8) as cd1,
        nc.sbuf_tensor("cd2", [P, FL], u8) as cd2,
        nc.sbuf_tensor("cd3", [P, FL], u8) as cd3,
        nc.sbuf_tensor("bias_sb", [P, 1], f32) as bias_sb,
        nc.semaphore("sem_in0") as sem_in0,
        nc.semaphore("sem_in1") as sem_in1,
        nc.semaphore("sem_in2") as sem_in2,
        nc.semaphore("sem_in3") as sem_in3,
        nc.semaphore("sem_m") as sem_m,
        nc.semaphore("sem_cd") as sem_cd,
        nc.semaphore("sem_out") as sem_out,
        nc.Block() as block,
    ):
        xv = x[:, :].rearrange("b (fh fl) -> (b fh) fl", fh=FH)
        ov = out[:, :, :].rearrange("b t (fh fl) -> t b fh fl", fh=FH)
        mts = [mt0, mt1, mt2, mt3]
        cds = [cd0, cd1, cd2, cd3]

        # pair p covers local steps 2p, 2p+1: q_even = g^(2p+1), q_odd = g^(2p+2)
        q_ev = [float(G ** ((2 * (p % NPAIR)) + 1)) for p in range(NP)]
        q_od = [float(G ** ((2 * (p % NPAIR)) + 2)) for p in range(NP)]
        last = NP - 1

        NQ = 4
        QW = FL // NQ
        sem_ins = [sem_in0, sem_in1, sem_in2, sem_in3]

        @block.sync
        def _(sync):
            for k in range(NQ):
                sync.dma_start(
                    out=x_sb[:, k * QW : (k + 1) * QW],
                    in_=xv[:, k * QW : (k + 1) * QW],
                ).then_inc(sem_ins[k], 16)
            for p in range(NP):
                sync.wait_ge(sem_cd, p + 1)
                sync.dma_start(
                    out=ov[p % NPAIR], in_=cds[p % RING][:, :]
                ).then_inc(sem_out, 16)

        @block.scalar
        def _(scalar):
            ACTF = mybir.ActivationFunctionType
            # dummy activation: pulls the exp table load off the critical
            # path (overlaps the input DMA / DVE setup)
            scalar.activation(
                cds[RING - 1][:, :1],
                mts[RING - 1][:, :1],
                ACTF.Exp,
                bias=bias_sb[:, :],
                scale=0.0,
            )
            for p in range(NP):
                scalar.wait_ge(sem_m, p + 2)
                if p >= RING:
                    scalar.wait_ge(sem_out, 16 * (p - (RING - 1)))
                scalar.activation(
                    cds[p % RING][:, :],
                    mts[(p + 1) % RING][:, :],
                    ACTF.Exp,
                    bias=bias_sb[:, :],
                    scale=-1.0 / q_od[p],
                )
                scalar.drain().then_inc(sem_cd, 1)

        @block.vector
        def _(vector):
            # setup: r = (0.5 - x) / x, m = 0 (consumed only by this engine)
            vector.memset(bias_sb[:, :], 2.2)
            for k in range(NQ):
                sl = slice(k * QW, (k + 1) * QW)
                vector.wait_ge(sem_ins[k], 16)
                vector.reciprocal_approx_fast(inv_sb[:, sl], x_sb[:, sl])
                vector._custom_dve(
                    nr_r,
                    out=r_sb[:, sl],
                    in0=x_sb[:, sl],
                    in1=inv_sb[:, sl],
                    s0=2.0,
                    s1=0.5,
                    imm2=3e38,
                )
            vector.drain()

            for p in range(NP):
                if p >= RING:
                    # mt[(p+1)%RING] was read by ACT at pair p-RING
                    vector.wait_ge(sem_cd, p - (RING - 1))
                if p % NPAIR == 0:
                    # zero-state pair: reads only r, no state tile needed
                    vector._custom_dve(
                        op0,
                        out=mts[(p + 1) % RING][:, :],
                        in0=r_sb[:, :],
                        s0=G,
                        s1=q_ev[p],
                        imm2=q_od[p],
                    ).then_inc(sem_m, 1)
                else:
                    vector._custom_dve(
                        op,
                        out=mts[(p + 1) % RING][:, :],
                        in0=mts[p % RING][:, :],
                        in1=r_sb[:, :],
                        s0=G,
                        s1=q_ev[p],
                        imm2=q_od[p],
                    ).then_inc(sem_m, 1)
            # sem_m fires at op completion (pre-drain); ACT therefore waits
            # one op deeper, and this trailing drain covers the last pair.
            vector.drain().then_inc(sem_m, 1)

    nc.finalize()
    return nc


def _get_nc(repeat: int = 1) -> bass.Bass:
    if repeat not in _cache:
        _cache[repeat] = _build(repeat)
    return _cache[repeat]


def _run(x: np.ndarray, repeat: int = 1):
    nc = _get_nc(repeat)
    shards = [
        np.ascontiguousarray(x[:, i * FS : (i + 1) * FS]) for i in range(NCORES)
    ]
    in_maps = [{"x": s} for s in shards]
    res = run_bass_kernel_spmd(nc, in_maps, core_ids=list(range(NCORES)))
    return [r["out"] for r in res.results]


def kernel(x: np.ndarray) -> np.ndarray:
    x = np.asarray(x, dtype=np.float32)
    outs = _run(x, repeat=1)
    code = np.concatenate(outs, axis=2)  # [B, NPAIR, F] uint8
    big = x >= 0.5  # r <= 0: spikes every step
    odd = code >= 6
    even = ((code > 2) & (code < 6)) | (odd & big[:, None, :])
    spikes = np.empty((B, T, F), dtype=np.float32)
    spikes[:, 0::2, :] = even
    spikes[:, 1::2, :] = odd
    return spikes


# revision 23
# speedup vs baseline: 4.0689x; 1.4190x over previous
"""Adaptive-threshold spike encoding on 8 TRN2 NeuronCores.

Math: the reference scans t=0..31 with
    acc += x; spike = acc >= thr_t; acc = spike ? 0 : acc; thr' = 0.9*thr + 0.1*|x|
With thr_t = x + 0.9^t*(0.5-x) (closed form), spike_t <=> acc_pre >= 0.9^t*(0.5-x)
where acc_pre = k*x (k = steps since last reset).  Dividing by x and scaling by
g^t (g = 1/0.9):  spike_t <=> m >= r  with  r = (0.5-x)/x  and  m = k*g^t,
updated as m' = select(m < r, g*m + q_t, 0),  q_t = g^(t+1).

TWO timesteps fuse into ONE custom-DVE op (one uop, 1x rate):
    M1 = select(m < r, g*m + q_t, 0);  m2 = select(M1 < r, g*M1 + q_t1, 0)
The pair state m2 three-way classifies both spike bits (for r > 0):
    m2 == 0        -> odd-step spike   (even 0, odd 1)
    m2 == q_t1     -> even-step spike  (M1 was reset; g*0+q_t1 is exact)
    m2 >= 2*q_t1   -> no spikes
(r <= 0, i.e. x >= 0.5, always has m2 == 0 and both bits set; host knows x.)
ScalarE compresses m2 to a uint8 code via Exp(-m2/q_t1 + 2.2):
m2=0 -> e^2.2=9.0 {9};  m2=q_t1 -> e^1.2=3.3 {3};  m2>=2q_t1 -> <=e^0.2 {0,1}
(large m2 decays to +0 — no overflow path).  Host decodes: odd = code>=6;
even = (2<code<6) | (odd & (x>=0.5)).

Sharding: feature dim across the 8 cores, 8192 features each, no comms.
"""

import numpy as np
from contextlib import ExitStack
import concourse.bass as bass
import concourse.bacc as bacc
import concourse.mybir as mybir
from concourse import dve_ops as _dve_ops
from concourse.dve_spec import (
    C0, C1, C2, Spec, Src0, Src1, Zero, select, lower, minn, _has_src1,
)
from concourse.dve_uop import DveOpSpec
from concourse.bass_utils import run_bass_kernel_spmd

B = 32
F = 65536
T = 32
NCORES = 8
FS = F // NCORES  # 8192 features per core
FH = 4
FL = FS // FH  # 2048
P = B * FH  # 128 partitions
NPAIR = T // 2  # 16 step-pairs

G = 1.0 / 0.9
RING = 6

_cache: dict = {}


def _register(name, body, reference):
    for op in _dve_ops.OPS:
        if op.name == name:
            return op
    spec = Spec(body=body, reference=reference)
    shas = {}
    for ver in ("v3", "v4"):
        uops = lower(spec, ver=ver)
        shas[ver] = DveOpSpec(
            name=name, opcode=0, uops=uops, rd1_en=_has_src1(spec)
        ).sha(ver)
    op = _dve_ops.DveOp(name, spec, subdim=False, uops_sha=shas)
    _dve_ops.OPS.append(op)
    _dve_ops.CUSTOM_DVE_SPECS[name] = op.spec
    _dve_ops._SUB_OPCODE_FOR_NAME[name] = (
        _dve_ops._CUSTOM_DVE_ROW_BASE + len(_dve_ops.OPS) - 1
    )
    return op


def _nr_r_op():
    # r = min((0.5 - x) * y1*(2 - x*y1), 3e38) — fused Newton step + (0.5-x)
    # mult; the min maps a NaN from an x==0 seed to "never spikes" (DVE
    # min/max pick the non-NaN operand).
    # in0 = x, in1 = y1 (seed recip), s0 = 2.0, s1 = 0.5, imm2 = 3e38
    return _register(
        "RECIP_NR_R2_ANT",
        minn((C1 - Src0) * ((C0 - Src0 * Src1) * Src1), C2),
        lambda in0, in1, s0, s1, imm2: np.minimum(
            np.nan_to_num(
                (np.float32(s1) - in0.astype(np.float32))
                * ((np.float32(s0) - in0 * in1) * in1),
                nan=np.float32(imm2),
            ),
            np.float32(imm2),
        ).astype(np.float32),
    )


def _first_pair_op():
    # pair from a zero state, reading only r: M1 = select(0 < r, q_t, 0);
    # out = select(M1 < r, g*M1 + q_t1, 0).  in0 = r, s0 = g, s1 = q_t,
    # imm2 = q_t1.
    M1 = select(Zero < Src0, C1, Zero)
    return _register(
        "SPIKE_FIRST_PAIR_ANT",
        select(M1 < Src0, M1 * C0 + C2, Zero),
        lambda in0, in1, s0, s1, imm2: (
            lambda M1: np.where(
                M1 < in0, M1 * np.float32(s0) + np.float32(imm2), 0.0
            )
        )(np.where(0.0 < in0, np.float32(s1), 0.0).astype(np.float32)).astype(
            np.float32
        ),
    )


def _state2_op():
    # in0 = m, in1 = r, s0 = g, s1 = q_t, imm2 = q_{t+1}
    S1 = select(Src0 < Src1, Src0 * C0 + C1, Zero)
    return _register(
        "SPIKE_STATE2_ANT",
        select(S1 < Src1, S1 * C0 + C2, Zero),
        lambda in0, in1, s0, s1, imm2: (
            lambda M1: np.where(M1 < in1, M1 * np.float32(s0) + np.float32(imm2), 0.0)
        )(
            np.where(
                in0 < in1, in0.astype(np.float32) * np.float32(s0) + np.float32(s1), 0.0
            ).astype(np.float32)
        ).astype(np.float32),
    )


def _build(repeat: int = 1) -> bass.Bass:
    f32 = mybir.dt.float32
    u8 = mybir.dt.uint8
    ALU = mybir.AluOpType
    op = _state2_op()
    nr_r = _nr_r_op()
    op0 = _first_pair_op()

    NP = NPAIR * repeat  # global step-pairs

    nc = bacc.Bacc(target_bir_lowering=False)
    x = nc.declare_dram_parameter("x", [B, FS], f32, isOutput=False)
    out = nc.declare_dram_parameter("out", [B, NPAIR, FS], u8, isOutput=True)

    f32_tiles = ["x_sb", "d_sb", "inv_sb", "r_sb",
                 "mt0", "mt1", "mt2", "mt3", "mt4", "mt5"]
    u8_tiles = ["cd0", "cd1", "cd2", "cd3", "cd4", "cd5"]
    sems = ["sem_in0", "sem_in1", "sem_in2", "sem_in3",
            "sem_m", "sem_cd", "sem_out"]
    with ExitStack() as ctx:
        tl = {n: ctx.enter_context(nc.sbuf_tensor(n, [P, FL], f32))
              for n in f32_tiles}
        tl.update({n: ctx.enter_context(nc.sbuf_tensor(n, [P, FL], u8))
                   for n in u8_tiles})
        bias_sb = ctx.enter_context(nc.sbuf_tensor("bias_sb", [P, 1], f32))
        sm = {n: ctx.enter_context(nc.semaphore(n)) for n in sems}
        x_sb, inv_sb, r_sb = tl["x_sb"], tl["inv_sb"], tl["r_sb"]
        sem_m, sem_cd, sem_out = sm["sem_m"], sm["sem_cd"], sm["sem_out"]
        block = ctx.enter_context(nc.Block())

        xv = x[:, :].rearrange("b (fh fl) -> (b fh) fl", fh=FH)
        ov = out[:, :, :].rearrange("b t (fh fl) -> t b fh fl", fh=FH)
        mts = [tl[f"mt{i}"] for i in range(RING)]
        cds = [tl[f"cd{i}"] for i in range(RING)]

        # pair p covers local steps 2p, 2p+1: q_even = g^(2p+1), q_odd = g^(2p+2)
        q_ev = [float(G ** ((2 * (p % NPAIR)) + 1)) for p in range(NP)]
        q_od = [float(G ** ((2 * (p % NPAIR)) + 2)) for p in range(NP)]
        last = NP - 1

        NQ = 4
        QW = FL // NQ
        sem_ins = [sm[f"sem_in{k}"] for k in range(NQ)]

        @block.sync
        def _(sync):
            for k in range(NQ):
                sync.dma_start(
                    out=x_sb[:, k * QW : (k + 1) * QW],
                    in_=xv[:, k * QW : (k + 1) * QW],
                ).then_inc(sem_ins[k], 16)
            for p in range(NP):
                sync.wait_ge(sem_cd, p + 1)
                sync.dma_start(
                    out=ov[p % NPAIR], in_=cds[p % RING][:, :]
                ).then_inc(sem_out, 16)

        @block.scalar
        def _(scalar):
            ACTF = mybir.ActivationFunctionType
            # dummy activation: pulls the exp table load off the critical
            # path (overlaps the input DMA / DVE setup)
            scalar.activation(
                cds[RING - 1][:, :1],
                mts[RING - 1][:, :1],
                ACTF.Exp,
                bias=bias_sb[:, :],
                scale=0.0,
            )
            for p in range(NP):
                scalar.wait_ge(sem_m, p + 2)
                if p >= RING:
                    scalar.wait_ge(sem_out, 16 * (p - (RING - 1)))
                scalar.activation(
                    cds[p % RING][:, :],
                    mts[(p + 1) % RING][:, :],
                    ACTF.Exp,
                    bias=bias_sb[:, :],
                    scale=-1.0 / q_od[p],
                )
                scalar.drain().then_inc(sem_cd, 1)

        @block.vector
        def _(vector):
            # setup: r = (0.5 - x) / x, m = 0 (consumed only by this engine)
            vector.memset(bias_sb[:, :], 2.2)
            for k in range(NQ):
                sl = slice(k * QW, (k + 1) * QW)
                vector.wait_ge(sem_ins[k], 16)
                vector.reciprocal_approx_fast(inv_sb[:, sl], x_sb[:, sl])
                vector._custom_dve(
                    nr_r,
                    out=r_sb[:, sl],
                    in0=x_sb[:, sl],
                    in1=inv_sb[:, sl],
                    s0=2.0,
                    s1=0.5,
                    imm2=3e38,
                )
            vector.drain()

            for p in range(NP):
                if p >= RING:
                    # mt[(p+1)%RING] was read by ACT at pair p-RING
                    vector.wait_ge(sem_cd, p - (RING - 1))
                if p % NPAIR == 0:
                    # zero-state pair: reads only r, no state tile needed
                    vector._custom_dve(
                        op0,
                        out=mts[(p + 1) % RING][:, :],
                        in0=r_sb[:, :],
                        s0=G,
                        s1=q_ev[p],
                        imm2=q_od[p],
                    ).then_inc(sem_m, 1)
                else:
                    vector._custom_dve(
                        op,
                        out=mts[(p + 1) % RING][:, :],
                        in0=mts[p % RING][:, :],
                        in1=r_sb[:, :],
                        s0=G,
                        s1=q_ev[p],
                        imm2=q_od[p],
                    ).then_inc(sem_m, 1)
            # sem_m fires at op completion (pre-drain); ACT therefore waits
            # one op deeper, and this trailing drain covers the last pair.
            vector.drain().then_inc(sem_m, 1)

    nc.finalize()
    return nc


def _get_nc(repeat: int = 1) -> bass.Bass:
    if repeat not in _cache:
        _cache[repeat] = _build(repeat)
    return _cache[repeat]


def _run(x: np.ndarray, repeat: int = 1):
    nc = _get_nc(repeat)
    shards = [
        np.ascontiguousarray(x[:, i * FS : (i + 1) * FS]) for i in range(NCORES)
    ]
    in_maps = [{"x": s} for s in shards]
    res = run_bass_kernel_spmd(nc, in_maps, core_ids=list(range(NCORES)))
    return [r["out"] for r in res.results]


def kernel(x: np.ndarray) -> np.ndarray:
    x = np.asarray(x, dtype=np.float32)
    outs = _run(x, repeat=1)
    code = np.concatenate(outs, axis=2)  # [B, NPAIR, F] uint8
    big = x >= 0.5  # r <= 0: spikes every step
    odd = code >= 6
    even = ((code > 2) & (code < 6)) | (odd & big[:, None, :])
    spikes = np.empty((B, T, F), dtype=np.float32)
    spikes[:, 0::2, :] = even
    spikes[:, 1::2, :] = odd
    return spikes
